# revision 1
# baseline (speedup 1.0000x reference)
"""Trainium2 Bass kernel for nn_DifferentiableTortuosity.

Math: 50 iterations of D = min(D, (conv4(D)/4 + 1) * ip) on a (B,512,512)
grid, sampled at start_coords. Information propagates 1 cell/iteration, so
D^50[start] depends only on cells within L1 distance 50 of start: a 101x101
window centered at start is exact. Out-of-map cells (window sticking past the
map edge) behave exactly like the reference's zero padding as long as they
start at D=0: eff >= 0 everywhere, so min keeps them pinned at 0.

Layout per core: 8 batch windows stacked along the free dim in 104-col slots
of one [101, 832] fp32 SBUF tile (3 zero guard cols between slots). Start is
always at local (50, 50+104*b), so one SPMD program serves all cores.

Per iteration (x50), on an active column band that shrinks with the
dependency cone (iteration k only influences the center from columns within
50-k, so the band narrows by 2 each iteration down to a single column):
  PE        : V = tridiag @ D        (vertical neighbor sum; fp32, bit-exact)
  DVE/Pool  : H4 = (left + 4) + right  (scalar_tensor_tensor)
  DVE       : N = V + H4             (PSUM + SBUF)
  DVE/Pool  : eff = N * (ip/4)       (== (conv4/4 + 1)*ip rounding in fp32)
  DVE/Pool  : D = min(D, eff)
The 8 batches split into two groups with independent dependency chains
interleaved across engines (PE/DVE/GpSimd) to hide semaphore latency.
"""
import numpy as np

B_FULL = 64
H = 512
W = 512
NCORES = 8
BPC = B_FULL // NCORES  # 8 batches per core
R = 50
WIN = 2 * R + 1   # 101
SLOT = 104        # window cols + 3 guard cols
WCOLS = SLOT * BPC  # 832
NUM_ITER = 50
EPS = 1e-06

_COMPILED = {}

# active-window floor (instruction overhead dominates below this width)
W_FLOOR = 25
# run eff (and min) on GpSimd while the active width is at least this
POOL_EFF_MIN_W = 41

# v3 config: number of batch groups and per-group engine picks
V3_GROUPS = 2
# NOTE: on real HW the Pool engine accepts tensor_tensor(mult) but rejects
# scalar_tensor_tensor and tensor_tensor(min) (walrus NCC_IXCG966 engine
# check), so only eff may run on Pool.
V3_EFF_ENG = ("pool", "pool")   # per group: "pool" | "dve"
V3_MIN_ENG = ("dve", "dve")
V3_H4_ENG = ("dve", "dve")
V3_FLOOR = 1
V3_PE_WARM = 0  # filler matmuls per iter to keep PE ramped
V3_MERGE_W = 0  # merge groups into one chain when w <= this (0 = never)


def _build_program_v3(n_iter=NUM_ITER):
    """Two independent batch-group chains interleaved across engines, with
    the active column band shrinking to the bare dependency cone (floor 1).
    Per group and iteration: PE tridiag matmul (V), DVE stt (H4=(l+4)+r),
    DVE add (N=V+H4), eff=N*ip4 and min on configurable engines."""
    import concourse.bacc as bacc
    import concourse.tile as tile
    from concourse import mybir

    nc = bacc.Bacc("TRN2", target_bir_lowering=False, debug=False,
                   num_devices=NCORES)
    pm_in = nc.declare_dram_parameter("pmwin", [WIN, WCOLS], mybir.dt.float32,
                                      isOutput=False)
    d0_in = nc.declare_dram_parameter("d0win", [WIN, WCOLS], mybir.dt.float32,
                                      isOutput=False)
    pl_out = nc.declare_dram_parameter("plens", [1, BPC], mybir.dt.float32,
                                       isOutput=True)
    warm_out = None
    if V3_PE_WARM:
        warm_out = nc.declare_dram_parameter(
            "warm_out", [1, 1], mybir.dt.float32, isOutput=True)

    tri_np = np.zeros((WIN, WIN), dtype=np.float32)
    for i in range(WIN):
        if i > 0:
            tri_np[i - 1, i] = 1.0
        if i < WIN - 1:
            tri_np[i + 1, i] = 1.0
    tri_dram = nc.inline_tensor(tri_np, "tri")

    G = V3_GROUPS
    GB = BPC // G  # batches per group

    GW = (BPC // G) * SLOT  # columns per group

    with tile.TileContext(nc) as tc:
        with (
            tc.tile_pool(name="state", bufs=1) as state,
            tc.tile_pool(name="tmp", bufs=3) as tmp,
            tc.tile_pool(name="ps", bufs=2, space="PSUM") as ps,
        ):
            D = state.tile([WIN, WCOLS], mybir.dt.float32)
            IP4 = state.tile([WIN, WCOLS], mybir.dt.float32)
            TRI = state.tile([WIN, WIN], mybir.dt.float32)
            nc.sync.dma_start(out=TRI[:], in_=tri_dram[:])
            nc.sync.dma_start(out=D[:], in_=d0_in[:])
            nc.sync.dma_start(out=IP4[:], in_=pm_in[:])
            nc.vector.tensor_scalar_add(IP4[:], IP4[:], float(EPS))
            nc.vector.reciprocal(IP4[:], IP4[:])
            nc.vector.tensor_scalar_mul(IP4[:], IP4[:], 0.25)
            Dv = D[:].rearrange("p (b s) -> p b s", s=SLOT)
            IPv = IP4[:].rearrange("p (b s) -> p b s", s=SLOT)

            def eng(name):
                return nc.gpsimd if name == "pool" else nc.vector

            WSB = None
            if V3_PE_WARM:
                WSB = state.tile([1, 1], mybir.dt.float32, tag="wsb")

            for it in range(1, n_iter + 1):
                w = max(WIN - 2 * it, V3_FLOOR)
                a = (WIN - w) // 2 + 1
                if V3_PE_WARM:
                    for _ in range(V3_PE_WARM):
                        PW = ps.tile([WIN, 64], mybir.dt.float32, tag="warm")
                        nc.tensor.matmul(PW[:], TRI[:], TRI[:, 0:64],
                                         start=True, stop=True)
                        if it == n_iter:
                            nc.vector.tensor_copy(WSB[:], PW[0:1, 0:1])
                merged = w <= V3_MERGE_W and BPC * w <= 512
                groups = [(0, BPC, "dve", "dve", "dve")] if merged else [
                    (g * GB, (g + 1) * GB,
                     V3_H4_ENG[g], V3_EFF_ENG[g], V3_MIN_ENG[g])
                    for g in range(G)]
                for gi, (b0, b1, h4e, effe, mine) in enumerate(groups):
                    if w < POOL_EFF_MIN_W:
                        effe = "dve"  # Pool ops at tiny widths are unproven
                    nb = b1 - b0
                    H4 = tmp.tile([WIN, nb * w], mybir.dt.float32,
                                  tag=f"h{gi}")
                    N = tmp.tile([WIN, nb * w], mybir.dt.float32,
                                 tag=f"n{gi}")
                    H4v = H4[:].rearrange("p (b s) -> p b s", s=w)
                    Nv = N[:].rearrange("p (b s) -> p b s", s=w)
                    PS0 = ps.tile([WIN, nb * w], mybir.dt.float32,
                                  tag=f"v{gi}")

                    eng(h4e).scalar_tensor_tensor(
                        H4v, Dv[:, b0:b1, a - 1:a - 1 + w], 4.0,
                        Dv[:, b0:b1, a + 1:a + 1 + w],
                        op0=mybir.AluOpType.add, op1=mybir.AluOpType.add)
                    nc.tensor.matmul(PS0[:], TRI[:],
                                     Dv[:, b0:b1, a:a + w],
                                     start=True, stop=True)
                    nc.vector.tensor_add(N[:], PS0[:], H4[:])
                    eng(effe).tensor_mul(Nv, Nv, IPv[:, b0:b1, a:a + w])
                    eng(mine).tensor_tensor(
                        Dv[:, b0:b1, a:a + w], Dv[:, b0:b1, a:a + w], Nv,
                        op=mybir.AluOpType.min)

            Dslots = D[:].rearrange("p (b s) -> p b s", s=SLOT)
            nc.sync.dma_start(out=pl_out[:],
                              in_=Dslots[R:R + 1, :, R:R + 1])
            if V3_PE_WARM:
                nc.sync.dma_start(out=warm_out[:], in_=WSB[:])

    nc.compile()
    return nc


def _prepare_core_inputs(pm, start, goal):
    """pm: (BPC,512,512) f32; start/goal: (BPC,2) int64 (already clipped).
    Returns pmwin, d0win tiles of shape (WIN, WCOLS)."""
    pmwin = np.ones((WIN, WCOLS), dtype=np.float32)
    d0win = np.zeros((WIN, WCOLS), dtype=np.float32)
    big = np.float32(H + W)
    for b in range(BPC):
        sr, sc = int(start[b, 0]), int(start[b, 1])
        r0, c0 = sr - R, sc - R
        rlo, rhi = max(0, r0), min(H, r0 + WIN)
        clo, chi = max(0, c0), min(W, c0 + WIN)
        cb = SLOT * b
        pmwin[rlo - r0:rhi - r0, cb + clo - c0:cb + chi - c0] = \
            pm[b, rlo:rhi, clo:chi]
        d0win[rlo - r0:rhi - r0, cb + clo - c0:cb + chi - c0] = big
        glr, glc = int(goal[b, 0]) - r0, int(goal[b, 1]) - c0
        if rlo - r0 <= glr < rhi - r0 and clo - c0 <= glc < chi - c0:
            d0win[glr, cb + glc] = 0.0
    return pmwin, d0win


def kernel(probability_map, start_coords, goal_coords, _trace=False,
           _n_iter=NUM_ITER):
    from concourse.bass_utils import run_bass_kernel_spmd

    pm = np.asarray(probability_map, dtype=np.float32)
    sc_all = np.asarray(start_coords)
    gc_all = np.asarray(goal_coords)
    B = pm.shape[0]
    assert pm.shape == (B_FULL, 1, H, W) and B == B_FULL

    sr = np.clip(sc_all[:, 0], 0, H - 1).astype(np.int64)
    sc = np.clip(sc_all[:, 1], 0, W - 1).astype(np.int64)
    gr = np.clip(gc_all[:, 0], 0, H - 1).astype(np.int64)
    gc = np.clip(gc_all[:, 1], 0, W - 1).astype(np.int64)
    start = np.stack([sr, sc], axis=1)
    goal = np.stack([gr, gc], axis=1)

    if _n_iter not in _COMPILED:
        _COMPILED[_n_iter] = _build_program_v3(_n_iter)
    nc = _COMPILED[_n_iter]

    in_maps = []
    for c in range(NCORES):
        lo = c * BPC
        pmwin, d0win = _prepare_core_inputs(
            pm[lo:lo + BPC, 0], start[lo:lo + BPC], goal[lo:lo + BPC])
        in_maps.append({"pmwin": pmwin, "d0win": d0win})

    res = run_bass_kernel_spmd(nc, in_maps, list(range(NCORES)))
    path_lengths = np.concatenate(
        [np.asarray(r["plens"]).reshape(BPC) for r in res.results])

    diff = (gc_all - sc_all).astype(np.float32)
    euclid = np.sqrt((diff * diff).sum(axis=1, dtype=np.float32))
    euclid = np.maximum(euclid, np.float32(1.0))
    tortuosity = (path_lengths / euclid).astype(np.float32)
    is_valid = path_lengths < np.float32(H + W)
    return tortuosity, is_valid



# revision 10
# speedup vs baseline: 1.6455x; 1.6455x over previous
"""Trainium2 Bass kernel for nn_DifferentiableTortuosity.

Math: 50 iterations of D = min(D, (conv4(D)/4 + 1) * ip) on a (B,512,512)
grid, sampled at start_coords. Information propagates 1 cell/iteration, so
D^50[start] depends only on cells within L1 distance 50 of start: a 101x101
window centered at start is exact. Out-of-map cells (window sticking past the
map edge) behave exactly like the reference's zero padding as long as they
start at D=0: eff >= 0 everywhere, so min keeps them pinned at 0.

Layout per core: 8 batch windows stacked along the free dim in 104-col slots
of one [101, 832] SBUF tile (3 zero guard cols between slots). Start is
always at local (50, 50+104*b), so one SPMD program serves all cores.

v4: bf16 state + full conv on PE. Per iteration (x50) on the shrinking
dependency-cone column band (w = 101-2*it):
  PE   : PS = TRI@D + I@D_left + I@D_right   (3 bf16 matmuls, fp32 PSUM)
  per group, either
    stt path : F = (PS + 4) * ip4            (DVE scalar_tensor_tensor)
    act path : E = Copy(PS + 4) -> bf16      (Act engine drains PSUM)
               F = E * ip4                   (Pool or DVE, bf16 2x mode)
  DVE  : D = min(D, F)                       (bf16 2x mode, one op per iter)
The per-engine busy budget is balanced DVE/Act/Pool/PE; bf16 halves DVE
tensor_tensor time (2x_1p) and makes the matmuls 1 cycle/row.
"""
import numpy as np
import ml_dtypes

BF = ml_dtypes.bfloat16

B_FULL = 64
H = 512
W = 512
NCORES = 8
BPC = B_FULL // NCORES  # 8 batches per core
R = 50
WIN = 2 * R + 1   # 101
SLOT = 104        # window cols + 3 guard cols
WCOLS = SLOT * BPC  # 832
NUM_ITER = 50
EPS = 1e-06

_COMPILED = {}

# --- v4 config knobs (swept offline, best found hardcoded) ---
POOL_MULT = True     # Act-path mult on Pool (True) or DVE (False)
# PLAN: list of (w_min, [(nb, strat), ...]) — first entry with w >= w_min
# wins. strat in {"stt", "act"}. Sum of nb must be BPC; nb*w <= 512.
PLAN = [
    (51, [(3, "stt"), (3, "stt"), (1, "act"), (1, "act")]),
    (0, [(3, "stt"), (3, "stt"), (2, "stt")]),
]
# One-time phase offset (ns) applied to group gi's first iteration, to run
# the independent batch-group chains in anti-phase instead of lockstep.
STAGGER_NS = 0
PS_BUFS = 2
TMP_BUFS = 3


def _group_plan(w):
    for w_min, sizes in PLAN:
        if w >= w_min:
            groups = []
            b0 = 0
            for nb, strat in sizes:
                groups.append((b0, b0 + nb, strat))
                b0 += nb
            assert b0 == BPC
            return groups
    raise AssertionError("no plan entry")


def _build_program_v4(n_iter=NUM_ITER):
    import concourse.bacc as bacc
    import concourse.tile as tile
    from concourse import mybir

    nc = bacc.Bacc("TRN2", target_bir_lowering=False, debug=False,
                   num_devices=NCORES)
    ip_in = nc.declare_dram_parameter("ip4win", [WIN, WCOLS],
                                      mybir.dt.bfloat16, isOutput=False)
    d0_in = nc.declare_dram_parameter("d0win", [WIN, WCOLS],
                                      mybir.dt.bfloat16, isOutput=False)
    pl_out = nc.declare_dram_parameter("plens", [1, BPC], mybir.dt.bfloat16,
                                       isOutput=True)

    tri_np = np.zeros((WIN, WIN), dtype=np.float32)
    for i in range(WIN):
        if i > 0:
            tri_np[i - 1, i] = 1.0
        if i < WIN - 1:
            tri_np[i + 1, i] = 1.0
    idn_np = np.eye(WIN, dtype=np.float32)
    tri_dram = nc.inline_tensor(tri_np.astype(BF), "tri")
    idn_dram = nc.inline_tensor(idn_np.astype(BF), "idn")

    with tile.TileContext(nc) as tc:
        with (
            tc.tile_pool(name="state", bufs=1) as state,
            tc.tile_pool(name="tmp", bufs=TMP_BUFS) as tmp,
            tc.tile_pool(name="ps", bufs=PS_BUFS, space="PSUM") as ps,
        ):
            D = state.tile([WIN, WCOLS], mybir.dt.bfloat16)
            IP4 = state.tile([WIN, WCOLS], mybir.dt.bfloat16)
            TRI = state.tile([WIN, WIN], mybir.dt.bfloat16)
            IDN = state.tile([WIN, WIN], mybir.dt.bfloat16)
            nc.sync.dma_start(out=TRI[:], in_=tri_dram[:])
            nc.sync.dma_start(out=IDN[:], in_=idn_dram[:])
            nc.sync.dma_start(out=D[:], in_=d0_in[:])
            nc.sync.dma_start(out=IP4[:], in_=ip_in[:])
            Dv = D[:].rearrange("p (b s) -> p b s", s=SLOT)
            IPv = IP4[:].rearrange("p (b s) -> p b s", s=SLOT)

            def emit_group(gi, b0, b1, strat, w, a):
                nb = b1 - b0
                PS = ps.tile([WIN, nb * w], mybir.dt.float32, tag=f"v{gi}")
                nc.tensor.matmul(PS[:], TRI[:], Dv[:, b0:b1, a:a + w],
                                 start=True, stop=False)
                nc.tensor.matmul(PS[:], IDN[:],
                                 Dv[:, b0:b1, a - 1:a - 1 + w],
                                 start=False, stop=False)
                nc.tensor.matmul(PS[:], IDN[:],
                                 Dv[:, b0:b1, a + 1:a + 1 + w],
                                 start=False, stop=True)
                PSv = PS[:].rearrange("p (b s) -> p b s", s=w)
                F = tmp.tile([WIN, nb * w], mybir.dt.bfloat16, tag=f"f{gi}")
                Fv = F[:].rearrange("p (b s) -> p b s", s=w)
                if strat == "stt":
                    nc.vector.scalar_tensor_tensor(
                        Fv, PSv, 4.0, IPv[:, b0:b1, a:a + w],
                        op0=mybir.AluOpType.add, op1=mybir.AluOpType.mult)
                else:
                    E = tmp.tile([WIN, nb * w], mybir.dt.bfloat16,
                                 tag=f"e{gi}")
                    nc.scalar.activation(
                        E[:], PS[:], mybir.ActivationFunctionType.Copy,
                        bias=4.0)
                    Ev = E[:].rearrange("p (b s) -> p b s", s=w)
                    eng = nc.gpsimd if POOL_MULT else nc.vector
                    eng.tensor_tensor(Fv, Ev, IPv[:, b0:b1, a:a + w],
                                      op=mybir.AluOpType.mult)
                nc.vector.tensor_tensor(
                    Dv[:, b0:b1, a:a + w], Dv[:, b0:b1, a:a + w],
                    Fv, op=mybir.AluOpType.min)

            for it in range(1, n_iter + 1):
                w = max(WIN - 2 * it, 1)
                a = (WIN - w) // 2 + 1
                groups = _group_plan(w)
                for gi, (b0, b1, strat) in enumerate(groups):
                    if it == 1 and gi > 0 and STAGGER_NS:
                        with tc.tile_wait_until(gi * STAGGER_NS * 1e-6):
                            emit_group(gi, b0, b1, strat, w, a)
                    else:
                        emit_group(gi, b0, b1, strat, w, a)

            Dslots = D[:].rearrange("p (b s) -> p b s", s=SLOT)
            nc.sync.dma_start(out=pl_out[:],
                              in_=Dslots[R:R + 1, :, R:R + 1])

    nc.compile()
    return nc


def _prepare_core_inputs(pm, start, goal):
    """pm: (BPC,512,512) f32; start/goal: (BPC,2) int (already clipped).
    Returns {"ip4win": bf16 (WIN,WCOLS), "d0win": bf16 (WIN,WCOLS)}."""
    pmwin = np.ones((WIN, WCOLS), dtype=np.float32)
    d0win = np.zeros((WIN, WCOLS), dtype=np.float32)
    big = np.float32(H + W)
    for b in range(BPC):
        sr, sc = int(start[b, 0]), int(start[b, 1])
        r0, c0 = sr - R, sc - R
        rlo, rhi = max(0, r0), min(H, r0 + WIN)
        clo, chi = max(0, c0), min(W, c0 + WIN)
        cb = SLOT * b
        pmwin[rlo - r0:rhi - r0, cb + clo - c0:cb + chi - c0] = \
            pm[b, rlo:rhi, clo:chi]
        d0win[rlo - r0:rhi - r0, cb + clo - c0:cb + chi - c0] = big
        glr, glc = int(goal[b, 0]) - r0, int(goal[b, 1]) - c0
        if rlo - r0 <= glr < rhi - r0 and clo - c0 <= glc < chi - c0:
            d0win[glr, cb + glc] = 0.0
    ip4win = (np.float32(0.25) *
              (np.float32(1.0) / (pmwin + np.float32(EPS)))).astype(BF)
    return {"ip4win": ip4win, "d0win": d0win.astype(BF)}


def kernel(probability_map, start_coords, goal_coords, _trace=False,
           _n_iter=NUM_ITER):
    from concourse.bass_utils import run_bass_kernel_spmd

    pm = np.asarray(probability_map, dtype=np.float32)
    sc_all = np.asarray(start_coords)
    gc_all = np.asarray(goal_coords)
    B = pm.shape[0]
    assert pm.shape == (B_FULL, 1, H, W) and B == B_FULL

    sr = np.clip(sc_all[:, 0], 0, H - 1).astype(np.int64)
    sc = np.clip(sc_all[:, 1], 0, W - 1).astype(np.int64)
    gr = np.clip(gc_all[:, 0], 0, H - 1).astype(np.int64)
    gc = np.clip(gc_all[:, 1], 0, W - 1).astype(np.int64)
    start = np.stack([sr, sc], axis=1)
    goal = np.stack([gr, gc], axis=1)

    if _n_iter not in _COMPILED:
        _COMPILED[_n_iter] = _build_program_v4(_n_iter)
    nc = _COMPILED[_n_iter]

    in_maps = []
    for c in range(NCORES):
        lo = c * BPC
        in_maps.append(_prepare_core_inputs(
            pm[lo:lo + BPC, 0], start[lo:lo + BPC], goal[lo:lo + BPC]))

    res = run_bass_kernel_spmd(nc, in_maps, list(range(NCORES)))
    path_lengths = np.concatenate(
        [np.asarray(r["plens"]).astype(np.float32).reshape(BPC)
         for r in res.results])

    diff = (gc_all - sc_all).astype(np.float32)
    euclid = np.sqrt((diff * diff).sum(axis=1, dtype=np.float32))
    euclid = np.maximum(euclid, np.float32(1.0))
    tortuosity = (path_lengths / euclid).astype(np.float32)
    is_valid = path_lengths < np.float32(H + W)
    return tortuosity, is_valid


# revision 25
# speedup vs baseline: 1.8274x; 1.1106x over previous
"""Trainium2 Bass kernel for nn_DifferentiableTortuosity.

Math: 50 iterations of D = min(D, (conv4(D)/4 + 1) * ip) on a (B,512,512)
grid, sampled at start_coords. Information propagates 1 cell/iteration, so
D^50[start] depends only on cells within L1 distance 50 of start: a 101x101
window centered at start is exact. Out-of-map cells (window sticking past the
map edge) behave exactly like the reference's zero padding as long as they
start at D=0: eff >= 0 everywhere, so min keeps them pinned at 0.

Layout per core: 8 batch windows stacked along the free dim in 104-col slots
of one [101, 832] SBUF tile (3 zero guard cols between slots). Start is
always at local (50, 50+104*b), so one SPMD program serves all cores.

v4: bf16 state + full conv on PE. Per iteration (x50) on the shrinking
dependency-cone column band (w = 101-2*it):
  PE   : PS = TRI@D + I@D_left + I@D_right   (3 bf16 matmuls, fp32 PSUM)
  per group, either
    stt path : F = (PS + 4) * ip4            (DVE scalar_tensor_tensor)
    act path : E = Copy(PS + 4) -> bf16      (Act engine drains PSUM)
               F = E * ip4                   (Pool or DVE, bf16 2x mode)
  DVE  : D = min(D, F)                       (bf16 2x mode, one op per iter)
The per-engine busy budget is balanced DVE/Act/Pool/PE; bf16 halves DVE
tensor_tensor time (2x_1p) and makes the matmuls 1 cycle/row.
"""
import numpy as np
import ml_dtypes

BF = ml_dtypes.bfloat16

B_FULL = 64
H = 512
W = 512
NCORES = 8
BPC = B_FULL // NCORES  # 8 batches per core
R = 50
WIN = 2 * R + 1   # 101
SLOT = 104        # window cols + 3 guard cols
WCOLS = SLOT * BPC  # 832
NUM_ITER = 50
EPS = 1e-06

_COMPILED = {}

# --- v4 config knobs (swept offline, best found hardcoded) ---
POOL_MULT = True     # Act-path mult on Pool (True) or DVE (False)
# PLAN: list of (w_min, [(nb, strat), ...]) — first entry with w >= w_min
# wins. strat in {"stt", "act"}. Sum of nb must be BPC; nb*w <= 512.
PLAN = [
    (31, [(2, "stt"), (2, "stt"), (2, "act"), (2, "act")]),
    (0, [(4, "stt"), (4, "stt")]),
]
# One-time phase offset (ns) applied to group gi's first iteration, to run
# the independent batch-group chains in anti-phase instead of lockstep.
STAGGER_NS = 0
PS_BUFS = 2
TMP_BUFS = 3


def _group_plan(w):
    for w_min, sizes in PLAN:
        if w >= w_min:
            groups = []
            b0 = 0
            for nb, strat in sizes:
                groups.append((b0, b0 + nb, strat))
                b0 += nb
            assert b0 == BPC
            return groups
    raise AssertionError("no plan entry")


def _build_program_v4(n_iter=NUM_ITER):
    import concourse.bacc as bacc
    import concourse.tile as tile
    from concourse import mybir

    nc = bacc.Bacc("TRN2", target_bir_lowering=False, debug=False,
                   num_devices=NCORES)
    # d0 windows (832 cols) + tridiag (101) + identity (101) in one DMA
    SCOLS = WCOLS + 2 * WIN
    st_in = nc.declare_dram_parameter("state0", [WIN, SCOLS],
                                      mybir.dt.bfloat16, isOutput=False)
    ip_in = nc.declare_dram_parameter("ip4win", [WIN, WCOLS],
                                      mybir.dt.bfloat16, isOutput=False)
    pl_out = nc.declare_dram_parameter("plens", [1, BPC], mybir.dt.bfloat16,
                                       isOutput=True)

    with tile.TileContext(nc) as tc:
        with (
            tc.tile_pool(name="state", bufs=1) as state,
            tc.tile_pool(name="tmp", bufs=TMP_BUFS) as tmp,
            tc.tile_pool(name="ps", bufs=PS_BUFS, space="PSUM") as ps,
        ):
            SD = state.tile([WIN, SCOLS], mybir.dt.bfloat16)
            IP4 = state.tile([WIN, WCOLS], mybir.dt.bfloat16)
            D = SD[:, 0:WCOLS]
            TRI = SD[:, WCOLS:WCOLS + WIN]
            IDN = SD[:, WCOLS + WIN:SCOLS]
            nc.sync.dma_start(out=SD[:], in_=st_in[:])
            nc.sync.dma_start(out=IP4[:], in_=ip_in[:])
            Dv = D.rearrange("p (b s) -> p b s", s=SLOT)
            IPv = IP4[:].rearrange("p (b s) -> p b s", s=SLOT)

            def emit_mms(b0, b1, w, a, ptag):
                nb = b1 - b0
                PS = ps.tile([WIN, nb * w], mybir.dt.float32, tag=ptag)
                nc.tensor.matmul(PS[:], TRI, Dv[:, b0:b1, a:a + w],
                                 start=True, stop=False)
                nc.tensor.matmul(PS[:], IDN,
                                 Dv[:, b0:b1, a - 1:a - 1 + w],
                                 start=False, stop=False)
                nc.tensor.matmul(PS[:], IDN,
                                 Dv[:, b0:b1, a + 1:a + 1 + w],
                                 start=False, stop=True)
                return PS

            def emit_min(b0, b1, Fv, w, a):
                nc.vector.tensor_tensor(
                    Dv[:, b0:b1, a:a + w], Dv[:, b0:b1, a:a + w],
                    Fv, op=mybir.AluOpType.min)

            def emit_stt_group(gi, b0, b1, w, a):
                nb = b1 - b0
                PS = emit_mms(b0, b1, w, a, f"v{gi}")
                PSv = PS[:].rearrange("p (b s) -> p b s", s=w)
                F = tmp.tile([WIN, nb * w], mybir.dt.bfloat16, tag=f"fs{gi}")
                Fv = F[:].rearrange("p (b s) -> p b s", s=w)
                nc.vector.scalar_tensor_tensor(
                    Fv, PSv, 4.0, IPv[:, b0:b1, a:a + w],
                    op0=mybir.AluOpType.add, op1=mybir.AluOpType.mult)
                emit_min(b0, b1, Fv, w, a)

            def emit_act_pair(gi, b0, b1, w, a, mult_eng):
                # two 1-batch conv triplets into halves of ONE single-bank
                # PSUM tile, then one drain + one mult + one min for the pair
                # (minimizes Act/DVE per-op fixed cost)
                nb = b1 - b0
                PS = ps.tile([WIN, nb * w], mybir.dt.float32, tag=f"va{gi}")
                for h in range(nb):
                    b = b0 + h
                    nc.tensor.matmul(PS[:, h * w:(h + 1) * w], TRI,
                                     Dv[:, b:b + 1, a:a + w],
                                     start=True, stop=False)
                    nc.tensor.matmul(PS[:, h * w:(h + 1) * w], IDN,
                                     Dv[:, b:b + 1, a - 1:a - 1 + w],
                                     start=False, stop=False)
                    nc.tensor.matmul(PS[:, h * w:(h + 1) * w], IDN,
                                     Dv[:, b:b + 1, a + 1:a + 1 + w],
                                     start=False, stop=True)
                F = tmp.tile([WIN, nb * w], mybir.dt.bfloat16, tag=f"fa{gi}")
                Fv = F[:].rearrange("p (b s) -> p b s", s=w)
                E = tmp.tile([WIN, nb * w], mybir.dt.bfloat16,
                             tag=f"ea{gi}")
                nc.scalar.activation(
                    E[:], PS[:], mybir.ActivationFunctionType.Copy,
                    bias=4.0)
                mult_eng.tensor_tensor(
                    Fv, E[:].rearrange("p (b s) -> p b s", s=w),
                    IPv[:, b0:b1, a:a + w], op=mybir.AluOpType.mult)
                emit_min(b0, b1, Fv, w, a)

            for it in range(1, n_iter + 1):
                w = max(WIN - 2 * it, 1)
                a = (WIN - w) // 2 + 1
                si = ai = 0
                for b0, b1, strat in _group_plan(w):
                    if strat == "stt":
                        emit_stt_group(si, b0, b1, w, a)
                        si += 1
                    else:
                        # "act" -> pool mult, "actd" -> DVE mult
                        eng = nc.vector if strat == "actd" else nc.gpsimd
                        emit_act_pair(ai, b0, b1, w, a, eng)
                        ai += 1

            Dslots = D.rearrange("p (b s) -> p b s", s=SLOT)
            nc.gpsimd.dma_start(out=pl_out[:],
                                in_=Dslots[R:R + 1, :, R:R + 1])

    nc.compile()
    return nc


def _tri_idn():
    tri = np.zeros((WIN, WIN), dtype=np.float32)
    for i in range(WIN):
        if i > 0:
            tri[i - 1, i] = 1.0
        if i < WIN - 1:
            tri[i + 1, i] = 1.0
    return tri, np.eye(WIN, dtype=np.float32)


def _prepare_core_inputs(pm, start, goal):
    """pm: (BPC,512,512) f32; start/goal: (BPC,2) int (already clipped).
    Returns {"state0": bf16 (WIN, WCOLS+2*WIN) = d0 windows | tridiag |
    identity, "ip4win": bf16 (WIN, WCOLS)}."""
    pmwin = np.ones((WIN, WCOLS), dtype=np.float32)
    state0 = np.zeros((WIN, WCOLS + 2 * WIN), dtype=np.float32)
    d0win = state0[:, 0:WCOLS]
    tri, idn = _tri_idn()
    state0[:, WCOLS:WCOLS + WIN] = tri
    state0[:, WCOLS + WIN:] = idn
    big = np.float32(H + W)
    for b in range(BPC):
        sr, sc = int(start[b, 0]), int(start[b, 1])
        r0, c0 = sr - R, sc - R
        rlo, rhi = max(0, r0), min(H, r0 + WIN)
        clo, chi = max(0, c0), min(W, c0 + WIN)
        cb = SLOT * b
        pmwin[rlo - r0:rhi - r0, cb + clo - c0:cb + chi - c0] = \
            pm[b, rlo:rhi, clo:chi]
        d0win[rlo - r0:rhi - r0, cb + clo - c0:cb + chi - c0] = big
        glr, glc = int(goal[b, 0]) - r0, int(goal[b, 1]) - c0
        if rlo - r0 <= glr < rhi - r0 and clo - c0 <= glc < chi - c0:
            d0win[glr, cb + glc] = 0.0
    ip4win = (np.float32(0.25) *
              (np.float32(1.0) / (pmwin + np.float32(EPS)))).astype(BF)
    return {"state0": state0.astype(BF), "ip4win": ip4win}


def kernel(probability_map, start_coords, goal_coords, _trace=False,
           _n_iter=NUM_ITER):
    from concourse.bass_utils import run_bass_kernel_spmd

    pm = np.asarray(probability_map, dtype=np.float32)
    sc_all = np.asarray(start_coords)
    gc_all = np.asarray(goal_coords)
    B = pm.shape[0]
    assert pm.shape == (B_FULL, 1, H, W) and B == B_FULL

    sr = np.clip(sc_all[:, 0], 0, H - 1).astype(np.int64)
    sc = np.clip(sc_all[:, 1], 0, W - 1).astype(np.int64)
    gr = np.clip(gc_all[:, 0], 0, H - 1).astype(np.int64)
    gc = np.clip(gc_all[:, 1], 0, W - 1).astype(np.int64)
    start = np.stack([sr, sc], axis=1)
    goal = np.stack([gr, gc], axis=1)

    if _n_iter not in _COMPILED:
        _COMPILED[_n_iter] = _build_program_v4(_n_iter)
    nc = _COMPILED[_n_iter]

    in_maps = []
    for c in range(NCORES):
        lo = c * BPC
        in_maps.append(_prepare_core_inputs(
            pm[lo:lo + BPC, 0], start[lo:lo + BPC], goal[lo:lo + BPC]))

    res = run_bass_kernel_spmd(nc, in_maps, list(range(NCORES)))
    path_lengths = np.concatenate(
        [np.asarray(r["plens"]).astype(np.float32).reshape(BPC)
         for r in res.results])

    diff = (gc_all - sc_all).astype(np.float32)
    euclid = np.sqrt((diff * diff).sum(axis=1, dtype=np.float32))
    euclid = np.maximum(euclid, np.float32(1.0))
    tortuosity = (path_lengths / euclid).astype(np.float32)
    is_valid = path_lengths < np.float32(H + W)
    return tortuosity, is_valid
